# revision 17
# baseline (speedup 1.0000x reference)
"""GQA attention kernel for 8 trn2 NeuronCores.

Sharding: core = (b, h) with b = core//4 (batch), h = core%4 (kv head).
Each core handles q heads 4h..4h+3 (a contiguous 512-column block of Wq),
its own kv head (128 rows of Wk/Wv), and the matching 512-column slice of
Wo.  Per-core output is a partial y (row-parallel Wo); host sums the 4
fp16 partials per batch in fp32.

All matmuls run in fp16 (full-rate at 2.4 GHz) with fp32 PSUM
accumulation.  The attention j-loop is exp-bound on the scalar engine,
so the q projection of head g+1 is software-pipelined into head g's
attention loop (one projection matmul per j iteration) to keep the PE
busy during exp waits.  Softmax normalization: an all-ones [128,128]
matmul gives the partition-broadcast key sum in one PE op, followed by
a full-width DVE approx reciprocal and multiply.

DMA priority order feeds the pipeline: wk, then x chunk 0 (n-split so
the first 256-wide k-projection sub-block starts as soon as its half
lands), wv, the rest of x, Wq, then Wo (only needed at the end).
"""

import numpy as np

EMB = 2048
N = 2048          # sequence length
HD = 128          # head dim
NHC = 4           # q heads per core
DQ = NHC * HD     # 512: per-core q concat dim
EC = 16           # e chunks of 128
SC = 16           # s chunks of 128
NB = 512          # n block size
NQ = 4            # n quarters in attention phase
SCALE = 1.0 / np.sqrt(HD)

_NC = None


def _build():
    import concourse.bass as bass
    from concourse import bacc
    import concourse.mybir as mybir
    import concourse.tile as tile
    from concourse.bass import ts

    FP32 = mybir.dt.float32
    F16 = mybir.dt.float16
    P = 128

    nc = bacc.Bacc("TRN2", target_bir_lowering=False, debug=False, num_devices=8)
    xT = nc.declare_dram_parameter("xT", [EMB, N], F16, isOutput=False)
    wqT = nc.declare_dram_parameter("wqT", [EMB, DQ], F16, isOutput=False)
    wkT = nc.declare_dram_parameter("wkT", [EMB, HD], F16, isOutput=False)
    wvT = nc.declare_dram_parameter("wvT", [EMB, HD], F16, isOutput=False)
    woT = nc.declare_dram_parameter("woT", [DQ, EMB], F16, isOutput=False)
    iden_d = nc.declare_dram_parameter("iden", [128, 128], F16, isOutput=False)
    ones_d = nc.declare_dram_parameter("ones", [128, 128], F16, isOutput=False)
    y = nc.declare_dram_parameter("y", [N, EMB], F16, isOutput=True)

    xT_r = xT[:].rearrange("(c p) n -> p c n", p=P)      # (128, 16, 2048)
    wqT_r = wqT[:].rearrange("(c p) d -> p c d", p=P)    # (128, 16, 512)
    wkT_r = wkT[:].rearrange("(c p) d -> p c d", p=P)    # (128, 16, 128)
    wvT_r = wvT[:].rearrange("(c p) d -> p c d", p=P)
    woT_r = woT[:].rearrange("(c p) e -> p c e", p=P)    # (128, 4, 2048)

    with tile.TileContext(nc) as tc:
      with tc.tile_pool(name="consts", bufs=1) as consts, \
           tc.tile_pool(name="persist", bufs=1) as persist:
        identity = consts.tile([P, P], F16, tag="identity")
        allones = consts.tile([P, P], F16, tag="allones")
        xc = [persist.tile([P, EC, NB], F16, tag=f"xc{nb}", name=f"xc{nb}")
              for nb in range(N // NB)]
        wk = persist.tile([P, EC, HD], F16, tag="wk")
        wv = persist.tile([P, EC, HD], F16, tag="wv")
        wq = persist.tile([P, EC, DQ], F16, tag="wq")
        wo = persist.tile([P, NHC, EMB], F16, tag="wo")

        # priority DMA order; x chunk 0 split along n so the first
        # k-projection sub-block only waits for the first 256 columns
        nc.sync.dma_start(wk[:], wkT_r)
        nc.sync.dma_start(wv[:], wvT_r)
        nc.sync.dma_start(xc[0][:, :, 0:256], xT_r[:, :, 0:256])
        nc.sync.dma_start(xc[0][:, :, 256:512], xT_r[:, :, 256:512])
        nc.sync.dma_start(wq[:, :, 0:128], wqT_r[:, :, 0:128])
        nc.sync.dma_start(identity[:], iden_d[:])
        nc.sync.dma_start(xc[1][:, 0:8, :], xT_r[:, 0:8, ts(1, NB)])
        nc.sync.dma_start(xc[1][:, 8:16, :], xT_r[:, 8:16, ts(1, NB)])
        nc.sync.dma_start(xc[2][:, 0:8, :], xT_r[:, 0:8, ts(2, NB)])
        nc.sync.dma_start(xc[2][:, 8:16, :], xT_r[:, 8:16, ts(2, NB)])
        nc.sync.dma_start(xc[3][:, 0:8, :], xT_r[:, 0:8, ts(3, NB)])
        nc.sync.dma_start(xc[3][:, 8:16, :], xT_r[:, 8:16, ts(3, NB)])
        nc.sync.dma_start(wq[:, :, 128:320], wqT_r[:, :, 128:320])
        nc.sync.dma_start(wq[:, :, 320:512], wqT_r[:, :, 320:512])
        nc.sync.dma_start(allones[:], ones_d[:])
        nc.sync.dma_start(wo[:, :, :], woT_r)

        kT = persist.tile([P, N], F16, tag="kT")
        V = persist.tile([P, SC, HD], F16, tag="V")
        qT = [persist.tile([P, N], F16, tag=f"qT{g}", name=f"qT{g}")
              for g in range(NHC)]
        OT = [persist.tile([P, N], F16, tag=f"OT{g}", name=f"OT{g}")
              for g in range(NHC)]

        # -------- k/v projections with head-0 q proj interleaved --------
        # first 512-block runs as two 256-wide sub-blocks to start sooner;
        # after each x chunk's k/v work, head 0's q projection for that
        # chunk runs (extra PE work per chunk absorbs DMA-feed hiccups)
        with tc.tile_pool(name="vTp", bufs=1) as vTp:
          vT = vTp.tile([P, N], F16, tag="vT")
          with tc.tile_pool(name="psA", bufs=3, space="PSUM") as psA, \
               tc.tile_pool(name="psA2", bufs=2, space="PSUM") as psA2, \
               tc.tile_pool(name="psT", bufs=1, space="PSUM") as psT, \
               tc.tile_pool(name="psQa", bufs=2, space="PSUM") as psQa:
            blocks = [(0, 0, 256), (0, 256, 256),
                      (1, 0, NB), (2, 0, NB), (3, 0, NB)]
            for nb, off, bw in blocks:
                base = nb * NB + off
                for t in range(2):
                    if bw == NB:
                        ps = psA.tile([P, NB], FP32, tag="psA",
                                      name=f"psKV_{base}_{t}")
                    else:
                        ps = psA2.tile([P, 256], FP32, tag="psA2",
                                       name=f"psKV_{base}_{t}")
                    w = wk if t == 0 else wv
                    for e in range(EC):
                        nc.tensor.matmul(
                            ps[:], w[:, e, :], xc[nb][:, e, ts(off // bw, bw)],
                            start=(e == 0), stop=(e == EC - 1),
                        )
                    if t == 0:
                        nc.scalar.copy(kT[:, base:base + bw], ps[:])
                    else:
                        nc.scalar.copy(vT[:, base:base + bw], ps[:])
                # transpose the freshly-written vT s-chunks into V (PE)
                for j in range(base // P, (base + bw) // P):
                    pt = psT.tile([P, P], F16, tag="psT", name=f"psT_{j}")
                    nc.tensor.transpose(pt[:], vT[:, ts(j, P)], identity[:])
                    nc.scalar.copy(V[:, j, :], pt[:])
                if off + bw == NB:    # head-0 q projection for this chunk
                    qs = psQa.tile([P, NB], FP32, tag="psQa",
                                   name=f"psQ0_{nb}")
                    for e in range(EC):
                        nc.tensor.matmul(
                            qs[:], wq[:, e, ts(0, HD)], xc[nb][:, e, :],
                            start=(e == 0), stop=(e == EC - 1),
                        )
                    nc.vector.tensor_copy(qT[0][:, ts(nb, NB)], qs[:])

          # ------------ attention with pipelined q projection ------------
          with tc.tile_pool(name="esp", bufs=3) as esp, \
               tc.tile_pool(name="lap", bufs=2) as lap, \
               tc.tile_pool(name="rbp", bufs=2) as rbp, \
               tc.tile_pool(name="psS", bufs=4, space="PSUM") as psS, \
               tc.tile_pool(name="psO", bufs=2, space="PSUM") as psO, \
               tc.tile_pool(name="psQ", bufs=2, space="PSUM") as psQ:

            def qproj_step(g, jj, ps_box):
                """One matmul of head g's q projection (jj in 0..63)."""
                nb, e = divmod(jj, EC)
                if e == 0:
                    ps_box[0] = psQ.tile([P, NB], FP32, tag="psQ",
                                         name=f"psQ_{g}_{nb}")
                nc.tensor.matmul(
                    ps_box[0][:], wq[:, e, ts(g, HD)], xc[nb][:, e, :],
                    start=(e == 0), stop=(e == EC - 1),
                )
                if e == EC - 1:
                    nc.vector.tensor_copy(qT[g][:, ts(nb, NB)], ps_box[0][:])

            qbox = [None]

            def finalize(g, m, lacc, ot_ps):
                # all-ones matmul: every partition gets the key-sum of
                # lacc -> reciprocal + normalize at full DVE width
                psl = psS.tile([P, NB], FP32, tag="psS", name=f"psl_{g}_{m}")
                nc.tensor.matmul(psl[:], allones[:], lacc[:],
                                 start=True, stop=True)
                rb = rbp.tile([P, NB], FP32, tag="rb", name=f"rb_{g}_{m}")
                nc.vector.reciprocal_approx_fast(rb[:], psl[:])
                nc.vector.tensor_mul(OT[g][:, ts(m, NB)], ot_ps[:], rb[:])

            pending = None    # (g, m, lacc, ot_ps) of the previous quarter
            for g in range(NHC):
                for m in range(NQ):
                    msl = ts(m, NB)
                    lacc = lap.tile([P, NB], F16, tag="lacc",
                                    name=f"lacc_{g}_{m}")
                    ot_ps = psO.tile([P, NB], FP32, tag="psO",
                                     name=f"psO_{g}_{m}")
                    for j in range(SC):
                        s_ps = psS.tile([P, NB], FP32, tag="psS",
                                        name=f"psS_{g}_{m}_{j}")
                        nc.tensor.matmul(
                            s_ps[:], kT[:, ts(j, P)], qT[g][:, msl],
                            start=True, stop=True,
                        )
                        if g < NHC - 1:
                            qproj_step(g + 1, m * SC + j, qbox)
                        es = esp.tile([P, NB], F16, tag="es",
                                      name=f"es_{g}_{m}_{j}")
                        nc.scalar.activation(
                            es[:], s_ps[:],
                            mybir.ActivationFunctionType.Exp,
                            scale=float(SCALE),
                        )
                        if j == 0:
                            nc.vector.tensor_copy(lacc[:], es[:])
                        else:
                            nc.vector.tensor_add(lacc[:], lacc[:], es[:])
                        nc.tensor.matmul(
                            ot_ps[:], V[:, j, :], es[:],
                            start=(j == 0), stop=(j == SC - 1),
                        )
                        if j == 1 and pending is not None:
                            finalize(*pending)   # overlap with this quarter
                            pending = None
                    pending = (g, m, lacc, ot_ps)
            finalize(*pending)

        # ---------------- output projection ----------------
        with tc.tile_pool(name="yep", bufs=2) as yep, \
             tc.tile_pool(name="psC", bufs=2, space="PSUM") as psC:
            for nt in range(N // P):
                yp = psC.tile([P, EMB], FP32, tag="psC", name=f"psC_{nt}")
                ysb = yep.tile([P, EMB], F16, tag="ysb", name=f"ysb_{nt}")
                if nt < N // P - 1:
                    for g in range(NHC):
                        lhsT = OT[g][:, ts(nt, P)]
                        for ob in range(4):
                            nc.tensor.matmul(
                                yp[:, ts(ob, NB)],
                                lhsT,
                                wo[:, g, ts(ob, NB)],
                                start=(g == 0), stop=(g == NHC - 1),
                            )
                    nc.scalar.copy(ysb[:], yp[:])
                    nc.sync.dma_start(y[ts(nt, P), :], ysb[:])
                else:
                    # last tile: per-quarter groups so copy/DMA overlap mms
                    for ob in range(4):
                        osl = ts(ob, NB)
                        for g in range(NHC):
                            nc.tensor.matmul(
                                yp[:, osl],
                                OT[g][:, ts(nt, P)],
                                wo[:, g, osl],
                                start=(g == 0), stop=(g == NHC - 1),
                            )
                        nc.scalar.copy(ysb[:, osl], yp[:, osl])
                        nc.sync.dma_start(y[ts(nt, P), osl], ysb[:, osl])

    nc.compile()
    return nc


def _in_maps(x, Wq, Wk, Wv, Wo):
    x = np.asarray(x, dtype=np.float32)
    Wq = np.asarray(Wq, dtype=np.float16)
    Wk = np.asarray(Wk, dtype=np.float16)
    Wv = np.asarray(Wv, dtype=np.float16)
    Wo = np.asarray(Wo, dtype=np.float16)
    xTs = [np.ascontiguousarray(x[b].T.astype(np.float16)) for b in range(2)]
    iden = np.eye(128, dtype=np.float16)
    ones = np.ones((128, 128), dtype=np.float16)
    maps = []
    for core in range(8):
        b, h = divmod(core, 4)
        maps.append({
            "xT": xTs[b],
            "wqT": np.ascontiguousarray(Wq[DQ * h:DQ * (h + 1), :].T),
            "wkT": np.ascontiguousarray(Wk[HD * h:HD * (h + 1), :].T),
            "wvT": np.ascontiguousarray(Wv[HD * h:HD * (h + 1), :].T),
            "woT": np.ascontiguousarray(Wo[:, DQ * h:DQ * (h + 1)].T),
            "iden": iden,
            "ones": ones,
        })
    return maps


def run(x, Wq, Wk, Wv, Wo, **spmd_kwargs):
    """Build/compile (cached) and run; returns BassKernelResults."""
    global _NC
    if _NC is None:
        _NC = _build()
    from concourse.bass_utils import run_bass_kernel_spmd
    return run_bass_kernel_spmd(_NC, _in_maps(x, Wq, Wk, Wv, Wo),
                                list(range(8)), **spmd_kwargs)


def kernel(x, attn_mask=None, is_causal=None, Wq=None, Wk=None, Wv=None,
           Wo=None, **_ignored):
    res = run(x, Wq, Wk, Wv, Wo)
    y = np.zeros((2, N, EMB), dtype=np.float32)
    for core in range(8):
        y[core // 4] += res.results[core]["y"].astype(np.float32)
    return y


# revision 18
# speedup vs baseline: 1.0230x; 1.0230x over previous
"""GQA attention kernel for 8 trn2 NeuronCores.

Sharding: core = (b, h) with b = core//4 (batch), h = core%4 (kv head).
Each core handles q heads 4h..4h+3 (a contiguous 512-column block of Wq),
its own kv head (128 rows of Wk/Wv), and the matching 512-column slice of
Wo.  Per-core output is a partial y (row-parallel Wo); host sums the 4
fp16 partials per batch in fp32.

All matmuls run in fp16 (full-rate at 2.4 GHz) with fp32 PSUM
accumulation.  The attention j-loop is exp-bound on the scalar engine,
so the q projection of head g+1 is software-pipelined into head g's
attention loop (one projection matmul per j iteration) to keep the PE
busy during exp waits.  Softmax normalization: an all-ones [128,128]
matmul gives the partition-broadcast key sum in one PE op, followed by
a full-width DVE approx reciprocal and multiply.

DMA priority order feeds the pipeline: wk, then x chunk 0 (n-split so
the first 256-wide k-projection sub-block starts as soon as its half
lands), wv, the rest of x, Wq, then Wo (only needed at the end).
"""

import numpy as np

EMB = 2048
N = 2048          # sequence length
HD = 128          # head dim
NHC = 4           # q heads per core
DQ = NHC * HD     # 512: per-core q concat dim
EC = 16           # e chunks of 128
SC = 16           # s chunks of 128
NB = 512          # n block size
NQ = 4            # n quarters in attention phase
SCALE = 1.0 / np.sqrt(HD)

_NC = None


def _build():
    import concourse.bass as bass
    from concourse import bacc
    import concourse.mybir as mybir
    import concourse.tile as tile
    from concourse.bass import ts

    FP32 = mybir.dt.float32
    F16 = mybir.dt.float16
    P = 128

    nc = bacc.Bacc("TRN2", target_bir_lowering=False, debug=False, num_devices=8)
    xT = nc.declare_dram_parameter("xT", [EMB, N], F16, isOutput=False)
    wqT = nc.declare_dram_parameter("wqT", [EMB, DQ], F16, isOutput=False)
    wkT = nc.declare_dram_parameter("wkT", [EMB, HD], F16, isOutput=False)
    wvT = nc.declare_dram_parameter("wvT", [EMB, HD], F16, isOutput=False)
    woT = nc.declare_dram_parameter("woT", [DQ, EMB], F16, isOutput=False)
    iden_d = nc.declare_dram_parameter("iden", [128, 128], F16, isOutput=False)
    ones_d = nc.declare_dram_parameter("ones", [128, 128], F16, isOutput=False)
    y = nc.declare_dram_parameter("y", [N, EMB], F16, isOutput=True)

    xT_r = xT[:].rearrange("(c p) n -> p c n", p=P)      # (128, 16, 2048)
    wqT_r = wqT[:].rearrange("(c p) d -> p c d", p=P)    # (128, 16, 512)
    wkT_r = wkT[:].rearrange("(c p) d -> p c d", p=P)    # (128, 16, 128)
    wvT_r = wvT[:].rearrange("(c p) d -> p c d", p=P)
    woT_r = woT[:].rearrange("(c p) e -> p c e", p=P)    # (128, 4, 2048)

    with tile.TileContext(nc) as tc:
      with tc.tile_pool(name="consts", bufs=1) as consts, \
           tc.tile_pool(name="persist", bufs=1) as persist:
        identity = consts.tile([P, P], F16, tag="identity")
        allones = consts.tile([P, P], F16, tag="allones")
        xc = [persist.tile([P, EC, NB], F16, tag=f"xc{nb}", name=f"xc{nb}")
              for nb in range(N // NB)]
        wk = persist.tile([P, EC, HD], F16, tag="wk")
        wv = persist.tile([P, EC, HD], F16, tag="wv")
        wq = persist.tile([P, EC, DQ], F16, tag="wq")
        wo = persist.tile([P, NHC, EMB], F16, tag="wo")

        # priority DMA order; x chunk 0 split along n so the first
        # k-projection sub-block only waits for the first 256 columns
        nc.sync.dma_start(wk[:], wkT_r)
        nc.sync.dma_start(wv[:], wvT_r)
        nc.sync.dma_start(xc[0][:, :, 0:256], xT_r[:, :, 0:256])
        nc.sync.dma_start(xc[0][:, :, 256:512], xT_r[:, :, 256:512])
        nc.sync.dma_start(wq[:, :, 0:128], wqT_r[:, :, 0:128])
        nc.sync.dma_start(identity[:], iden_d[:])
        nc.sync.dma_start(xc[1][:, 0:8, :], xT_r[:, 0:8, ts(1, NB)])
        nc.sync.dma_start(xc[1][:, 8:16, :], xT_r[:, 8:16, ts(1, NB)])
        nc.sync.dma_start(xc[2][:, 0:8, :], xT_r[:, 0:8, ts(2, NB)])
        nc.sync.dma_start(xc[2][:, 8:16, :], xT_r[:, 8:16, ts(2, NB)])
        nc.sync.dma_start(xc[3][:, 0:8, :], xT_r[:, 0:8, ts(3, NB)])
        nc.sync.dma_start(xc[3][:, 8:16, :], xT_r[:, 8:16, ts(3, NB)])
        nc.sync.dma_start(wq[:, :, 128:320], wqT_r[:, :, 128:320])
        nc.sync.dma_start(wq[:, :, 320:512], wqT_r[:, :, 320:512])
        nc.sync.dma_start(allones[:], ones_d[:])
        nc.sync.dma_start(wo[:, :, :], woT_r)

        kT = persist.tile([P, N], F16, tag="kT")
        V = persist.tile([P, SC, HD], F16, tag="V")
        qT = [persist.tile([P, N], F16, tag=f"qT{g}", name=f"qT{g}")
              for g in range(NHC)]
        OT = [persist.tile([P, N], F16, tag=f"OT{g}", name=f"OT{g}")
              for g in range(NHC)]

        # -------- k/v projections with head-0 q proj interleaved --------
        # first 512-block runs as two 256-wide sub-blocks to start sooner;
        # after each x chunk's k/v work, head 0's q projection for that
        # chunk runs (extra PE work per chunk absorbs DMA-feed hiccups)
        with tc.tile_pool(name="vTp", bufs=1) as vTp:
          vT = vTp.tile([P, N], F16, tag="vT")
          with tc.tile_pool(name="psA", bufs=3, space="PSUM") as psA, \
               tc.tile_pool(name="psA2", bufs=2, space="PSUM") as psA2, \
               tc.tile_pool(name="psT", bufs=1, space="PSUM") as psT, \
               tc.tile_pool(name="psQa", bufs=2, space="PSUM") as psQa:
            blocks = [(0, 0, 256), (0, 256, 256),
                      (1, 0, NB), (2, 0, NB), (3, 0, NB)]
            for nb, off, bw in blocks:
                base = nb * NB + off
                for t in range(2):
                    if bw == NB:
                        ps = psA.tile([P, NB], FP32, tag="psA",
                                      name=f"psKV_{base}_{t}")
                    else:
                        ps = psA2.tile([P, 256], FP32, tag="psA2",
                                       name=f"psKV_{base}_{t}")
                    w = wk if t == 0 else wv
                    for e in range(EC):
                        nc.tensor.matmul(
                            ps[:], w[:, e, :], xc[nb][:, e, ts(off // bw, bw)],
                            start=(e == 0), stop=(e == EC - 1),
                        )
                    if t == 0:
                        nc.scalar.copy(kT[:, base:base + bw], ps[:])
                    else:
                        nc.scalar.copy(vT[:, base:base + bw], ps[:])
                # transpose the freshly-written vT s-chunks into V (PE)
                for j in range(base // P, (base + bw) // P):
                    pt = psT.tile([P, P], F16, tag="psT", name=f"psT_{j}")
                    nc.tensor.transpose(pt[:], vT[:, ts(j, P)], identity[:])
                    nc.scalar.copy(V[:, j, :], pt[:])
                if off + bw == NB:    # head-0 q projection for this chunk
                    qs = psQa.tile([P, NB], FP32, tag="psQa",
                                   name=f"psQ0_{nb}")
                    for e in range(EC):
                        nc.tensor.matmul(
                            qs[:], wq[:, e, ts(0, HD)], xc[nb][:, e, :],
                            start=(e == 0), stop=(e == EC - 1),
                        )
                    nc.vector.tensor_copy(qT[0][:, ts(nb, NB)], qs[:])

          # ------------ attention with pipelined q projection ------------
          with tc.tile_pool(name="esp", bufs=3) as esp, \
               tc.tile_pool(name="lap", bufs=2) as lap, \
               tc.tile_pool(name="rbp", bufs=2) as rbp, \
               tc.tile_pool(name="psS", bufs=4, space="PSUM") as psS, \
               tc.tile_pool(name="psO", bufs=2, space="PSUM") as psO, \
               tc.tile_pool(name="psQ", bufs=2, space="PSUM") as psQ:

            def qproj_step(g, jj, ps_box):
                """One matmul of head g's q projection (jj in 0..63)."""
                nb, e = divmod(jj, EC)
                if e == 0:
                    ps_box[0] = psQ.tile([P, NB], FP32, tag="psQ",
                                         name=f"psQ_{g}_{nb}")
                nc.tensor.matmul(
                    ps_box[0][:], wq[:, e, ts(g, HD)], xc[nb][:, e, :],
                    start=(e == 0), stop=(e == EC - 1),
                )
                if e == EC - 1:
                    nc.vector.tensor_copy(qT[g][:, ts(nb, NB)], ps_box[0][:])

            qbox = [None]

            def finalize(g, m, lacc, ot_ps):
                # all-ones matmul: every partition gets the key-sum of
                # lacc -> reciprocal + normalize at full DVE width
                pool = psQ if g == NHC - 1 else psS
                psl = pool.tile([P, NB], FP32,
                                tag="psQ" if g == NHC - 1 else "psS",
                                name=f"psl_{g}_{m}")
                nc.tensor.matmul(psl[:], allones[:], lacc[:],
                                 start=True, stop=True)
                rb = rbp.tile([P, NB], FP32, tag="rb", name=f"rb_{g}_{m}")
                nc.vector.reciprocal_approx_fast(rb[:], psl[:])
                nc.vector.tensor_mul(OT[g][:, ts(m, NB)], ot_ps[:], rb[:])

            pending = None    # (g, m, lacc, ot_ps) of the previous quarter
            for g in range(NHC):
                for m in range(NQ):
                    msl = ts(m, NB)
                    lacc = lap.tile([P, NB], F16, tag="lacc",
                                    name=f"lacc_{g}_{m}")
                    ot_ps = psO.tile([P, NB], FP32, tag="psO",
                                     name=f"psO_{g}_{m}")
                    for j in range(SC):
                        s_ps = psS.tile([P, NB], FP32, tag="psS",
                                        name=f"psS_{g}_{m}_{j}")
                        nc.tensor.matmul(
                            s_ps[:], kT[:, ts(j, P)], qT[g][:, msl],
                            start=True, stop=True,
                        )
                        if g < NHC - 1:
                            qproj_step(g + 1, m * SC + j, qbox)
                        es = esp.tile([P, NB], F16, tag="es",
                                      name=f"es_{g}_{m}_{j}")
                        nc.scalar.activation(
                            es[:], s_ps[:],
                            mybir.ActivationFunctionType.Exp,
                            scale=float(SCALE),
                        )
                        if j == 0:
                            nc.vector.tensor_copy(lacc[:], es[:])
                        else:
                            nc.vector.tensor_add(lacc[:], lacc[:], es[:])
                        nc.tensor.matmul(
                            ot_ps[:], V[:, j, :], es[:],
                            start=(j == 0), stop=(j == SC - 1),
                        )
                        if j == 1 and pending is not None:
                            finalize(*pending)   # overlap with this quarter
                            pending = None
                    pending = (g, m, lacc, ot_ps)
            finalize(*pending)

        # ---------------- output projection ----------------
        with tc.tile_pool(name="yep", bufs=4) as yep, \
             tc.tile_pool(name="psC", bufs=4, space="PSUM") as psC:
            HM = EMB // 2
            for nt in range(N // P):
                for h in range(2):
                    yp = psC.tile([P, HM], FP32, tag="psC",
                                  name=f"psC_{nt}_{h}")
                    ysb = yep.tile([P, HM], F16, tag="ysb",
                                   name=f"ysb_{nt}_{h}")
                    for g in range(NHC):
                        lhsT = OT[g][:, ts(nt, P)]
                        for ob in range(2):
                            nc.tensor.matmul(
                                yp[:, ts(ob, NB)],
                                lhsT,
                                wo[:, g, ts(2 * h + ob, NB)],
                                start=(g == 0), stop=(g == NHC - 1),
                            )
                    nc.scalar.copy(ysb[:], yp[:])
                    nc.sync.dma_start(y[ts(nt, P), ts(h, HM)], ysb[:])

    nc.compile()
    return nc


def _in_maps(x, Wq, Wk, Wv, Wo):
    x = np.asarray(x, dtype=np.float32)
    Wq = np.asarray(Wq, dtype=np.float16)
    Wk = np.asarray(Wk, dtype=np.float16)
    Wv = np.asarray(Wv, dtype=np.float16)
    Wo = np.asarray(Wo, dtype=np.float16)
    xTs = [np.ascontiguousarray(x[b].T.astype(np.float16)) for b in range(2)]
    iden = np.eye(128, dtype=np.float16)
    ones = np.ones((128, 128), dtype=np.float16)
    maps = []
    for core in range(8):
        b, h = divmod(core, 4)
        maps.append({
            "xT": xTs[b],
            "wqT": np.ascontiguousarray(Wq[DQ * h:DQ * (h + 1), :].T),
            "wkT": np.ascontiguousarray(Wk[HD * h:HD * (h + 1), :].T),
            "wvT": np.ascontiguousarray(Wv[HD * h:HD * (h + 1), :].T),
            "woT": np.ascontiguousarray(Wo[:, DQ * h:DQ * (h + 1)].T),
            "iden": iden,
            "ones": ones,
        })
    return maps


def run(x, Wq, Wk, Wv, Wo, **spmd_kwargs):
    """Build/compile (cached) and run; returns BassKernelResults."""
    global _NC
    if _NC is None:
        _NC = _build()
    from concourse.bass_utils import run_bass_kernel_spmd
    return run_bass_kernel_spmd(_NC, _in_maps(x, Wq, Wk, Wv, Wo),
                                list(range(8)), **spmd_kwargs)


def kernel(x, attn_mask=None, is_causal=None, Wq=None, Wk=None, Wv=None,
           Wo=None, **_ignored):
    res = run(x, Wq, Wk, Wv, Wo)
    y = np.zeros((2, N, EMB), dtype=np.float32)
    for core in range(8):
        y[core // 4] += res.results[core]["y"].astype(np.float32)
    return y
